# revision 29
# baseline (speedup 1.0000x reference)
"""FM (factorization machine) forward kernel for Trainium2, 8-core data parallel.

Reference computation (per batch row b with field indices x[b, 0..3]):
    xo      = x + field_offsets                      # global rows into tables
    e_f     = v[xo_f]        (16-dim embedding)      # per-field lookup
    bias_f  = bias[xo_f]     (scalar)
    s       = sum_f e_f ;  q = sum_f e_f^2
    y       = sigmoid( sum_f bias_f + 0.5 * sum_k (s_k^2 - q_k) )

Sharding: batch 4096 split across 8 cores (512 rows each); the lookup table is
replicated.

Device-side design (per core, per iteration):
  * 1536 table rows x 256B gathered from ONE combined table as TWO SWDGE
    dma_gather calls of 768 idxs on separate SWDGE queues (the ~1us fixed
    descriptor-gen overhead is per call — fewer calls is better — but this
    runtime hard-fails on chunks > ~1000 descs / non-default DMA scratch,
    and a single queue stalls on 1024-desc ring reclaim).  int16 index
    space is tight, so user entries (17 f32: v16+bias) are packed two per
    64-f32 row; a 3-op DVE select picks the half via a host-shipped mask.
       rows [0, 15680)          user pairs   [e_2r (32 f32) | e_2r+1]
       rows [15680, 22487)      item         [v(16) | bias | 0...]
       rows [22487, 24179)      genre-year   [v_g+v_y | b_g+b_y | v_g^2+v_y^2]
  * DVE: select + sums/squares + reduce (self-sem hazard wait) -> logit.
  * ACT: sigmoid; sync engine: input loads + lagged output stores.
  * 4-slot software pipeline across repeat iterations: loads, descriptor
    gen, gather DMA, DVE math, and sigmoid/store all overlap; steady state
    (~2.7us/iter) sits near the gather's DMA bus floor (1536 descs x
    256B read-modify-write-penalized transfers over 16 DMA engines).
"""

import numpy as np

N_CORES = 8
BATCH = 4096
ROWS = BATCH // N_CORES          # 512 rows per core
P = 128                          # SBUF partitions
T = ROWS // P                    # 4 batch tiles per core
K = 16                           # embedding dim
ELEM = 64                        # table row (f32) -> 256B, dma_gather granule
VU, VI, VG, VY = 31360, 6807, 18, 94
VM = VG * VY                     # 1692 merged genre-year combos
NU2 = VU // 2                    # 15680 packed user rows
OFF_I = NU2                      # item rows start
OFF_M = NU2 + VI                 # 22487 merged rows start
NTAB = OFF_M + VM                # 24179 total rows (< int16 max)
NIDX = 3 * ROWS                  # 1536 gathered rows per iteration
IDXC = NIDX // 16                # 96 idx cols (16-partition wrap)
AUXW = IDXC // 2 + T * 17        # 48 f32 (96 i16 idxs) + 68 f32 (h17 mask)
S = 4                            # pipeline depth (buffer slots)
NS = 2                           # gather split (1536/NS idxs per dma_gather)
NQ = NS                          # one SWDGE queue (own 1024-desc ring) per chunk
DETECT_RACES = True              # sim-only knob (HW path ignores it)

_CACHE = {}


def _build(repeat=1):
    """Single-core Bass program (same program SPMD on all cores).

    repeat > 1 unrolls the body for steady-state timing with a 4-slot
    software pipeline (aux loads and gathers run ahead of the math).
    """
    from contextlib import ExitStack

    import concourse.bacc as bacc
    import concourse.mybir as mybir
    from concourse.library_config import mlp

    # one SWDGE queue per gather chunk: each queue has its own 1024-desc
    # descriptor ring, so iteration r+1's descriptor generation is not
    # stalled on reclaiming iteration r's in-flight transfers
    # NOTE: chunks > ~1000 descriptors or dynamic_dma_scratch_size != default
    # hard-fail on this runtime (NRT carveout is fixed) — keep chunks <= 768
    # and the default 16KB scratch.  Two queues measured faster than one
    # queue (ring-reclaim stalls) and than four alternating queues.
    nc = bacc.Bacc(
        "TRN2",
        debug=False,
        detect_race_conditions=DETECT_RACES,
        num_swdge_queues=NQ,
    )
    f32 = mybir.dt.float32
    i16 = mybir.dt.int16
    OP = mybir.AluOpType
    AF = mybir.ActivationFunctionType

    idx_d = nc.dram_tensor("idx16", [P, IDXC], i16, kind="ExternalInput")
    h_d = nc.dram_tensor("h17", [P, T * 17], f32, kind="ExternalInput")
    tab_d = nc.dram_tensor("tab", [NTAB, ELEM], f32, kind="ExternalInput")
    out_d = nc.dram_tensor("out", [P, T], f32, kind="ExternalOutput")

    GW = 12 * ELEM                   # gathered row block per slot (12 tiles)
    HW_ = T * 17                     # h17 mask width

    with ExitStack() as ctx:
        idxsb = ctx.enter_context(nc.sbuf_tensor([P, S * IDXC], i16))
        hsb = ctx.enter_context(nc.sbuf_tensor([P, S * HW_], f32))
        g = ctx.enter_context(nc.sbuf_tensor([P, S * GW], f32))
        z = ctx.enter_context(nc.sbuf_tensor([P, S * T], f32))
        y = ctx.enter_context(nc.sbuf_tensor([P, S * T], f32))
        d17 = ctx.enter_context(nc.sbuf_tensor([P, T * 17], f32))
        hd = ctx.enter_context(nc.sbuf_tensor([P, T * 17], f32))
        eu = ctx.enter_context(nc.sbuf_tensor([P, T * 17], f32))
        t17 = ctx.enter_context(nc.sbuf_tensor([P, T * 17], f32))
        s17 = ctx.enter_context(nc.sbuf_tensor([P, T * 17], f32))
        squ = ctx.enter_context(nc.sbuf_tensor([P, T * K], f32))
        sqi = ctx.enter_context(nc.sbuf_tensor([P, T * K], f32))
        qa = ctx.enter_context(nc.sbuf_tensor([P, T * K], f32))
        q16 = ctx.enter_context(nc.sbuf_tensor([P, T * K], f32))
        s2 = ctx.enter_context(nc.sbuf_tensor([P, T * K], f32))
        dd = ctx.enter_context(nc.sbuf_tensor([P, T * K], f32))
        rv = ctx.enter_context(nc.sbuf_tensor([P, T], f32))
        idxld = ctx.enter_context(nc.semaphore("idxld"))
        # one DMA-completion sem per SWDGE queue (sems are queue-locked)
        dg = [
            ctx.enter_context(nc.semaphore(f"dg{j}")) for j in range(NQ)
        ]
        gdone = ctx.enter_context(nc.semaphore("gdone"))
        sv = ctx.enter_context(nc.semaphore("sv"))
        svr = ctx.enter_context(nc.semaphore("svr"))
        sa = ctx.enter_context(nc.semaphore("sa"))
        outd = ctx.enter_context(nc.semaphore("outd"))
        block = ctx.enter_context(nc.Block())

        def gv(s_):
            return g[:, s_ * GW:(s_ + 1) * GW].rearrange(
                "p (t k) -> p t k", t=12, k=ELEM
            )

        @block.sync
        def _(sync):
            for r in range(repeat):
                s_ = r % S
                if r >= S:
                    sync.wait_ge(gdone, r - (S - 1))  # DVE freed slot r-S
                sync.dma_start(
                    out=idxsb[:, s_ * IDXC:(s_ + 1) * IDXC], in_=idx_d[:]
                ).then_inc(idxld, 16)
                sync.dma_start(
                    out=hsb[:, s_ * HW_:(s_ + 1) * HW_], in_=h_d[:]
                ).then_inc(idxld, 16)
                # store for iteration r-2 (lags so loads can run ahead)
                if r >= 2:
                    sync.wait_ge(sa, r - 1)           # sigmoid r-2 done
                    sync.dma_start(
                        out=out_d[:],
                        in_=y[:, ((r - 2) % S) * T:((r - 2) % S + 1) * T],
                    ).then_inc(outd, 16)
            for rr in (repeat - 2, repeat - 1):       # drain last stores
                if rr >= 0:
                    sync.wait_ge(sa, rr + 1)
                    sync.dma_start(
                        out=out_d[:],
                        in_=y[:, (rr % S) * T:(rr % S + 1) * T],
                    ).then_inc(outd, 16)
            sync.wait_ge(outd, 16 * repeat)  # one store per iteration total

        @block.gpsimd
        def _(gpsimd):
            gpsimd.load_library(mlp)
            nsub = NIDX // NS                    # idxs per gather chunk
            tsub = nsub // P                     # output tiles per chunk
            csub = nsub // 16                    # idx i16 cols per chunk
            nreg = gpsimd.to_reg(nsub)
            for r in range(repeat):
                s_ = r % S
                gpsimd.wait_ge(idxld, 32 * (r + 1))
                if r >= S:
                    gpsimd.wait_ge(gdone, r - (S - 1))
                for j in range(NS):
                    qn = j
                    gpsimd.dma_gather(
                        out_ap=gv(s_)[:, j * tsub:(j + 1) * tsub, :],
                        in_ap=tab_d[:],
                        idxs_ap=idxsb[
                            :, s_ * IDXC + j * csub:s_ * IDXC + (j + 1) * csub
                        ],
                        num_idxs=nsub,
                        num_idxs_reg=nreg,
                        elem_size=ELEM,
                        queue_num=qn,
                    ).then_inc(dg[qn], 16)

        @block.vector
        def _(vector):
            d3 = d17[:].rearrange("p (t k) -> p t k", t=T, k=17)
            hd3 = hd[:].rearrange("p (t k) -> p t k", t=T, k=17)
            eu3 = eu[:].rearrange("p (t k) -> p t k", t=T, k=17)
            t3 = t17[:].rearrange("p (t k) -> p t k", t=T, k=17)
            s3 = s17[:].rearrange("p (t k) -> p t k", t=T, k=17)
            squ3 = squ[:].rearrange("p (t k) -> p t k", t=T, k=K)
            sqi3 = sqi[:].rearrange("p (t k) -> p t k", t=T, k=K)
            qa3 = qa[:].rearrange("p (t k) -> p t k", t=T, k=K)
            q163 = q16[:].rearrange("p (t k) -> p t k", t=T, k=K)
            s23 = s2[:].rearrange("p (t k) -> p t k", t=T, k=K)
            dd3 = dd[:].rearrange("p (t k) -> p t k", t=T, k=K)
            for r in range(repeat):
                s_ = r % S
                g3 = gv(s_)
                gu_lo = g3[:, 0:T, 0:17]
                gu_hi = g3[:, 0:T, 32:49]
                gi17 = g3[:, T:2 * T, 0:17]
                gi16 = g3[:, T:2 * T, 0:K]
                gm_s17 = g3[:, 2 * T:3 * T, 0:17]
                gm_q16 = g3[:, 2 * T:3 * T, 17:17 + K]
                h173 = hsb[:, s_ * HW_:(s_ + 1) * HW_].rearrange(
                    "p (t k) -> p t k", t=T, k=17
                )
                rv3 = rv[:].rearrange("p (t o) -> p t o", t=T, o=1)
                zv = z[:, s_ * T:(s_ + 1) * T].rearrange(
                    "p (t o) -> p t o", t=T, o=1
                )
                for j in range(NS):
                    vector.wait_ge(dg[j], 16 * (r + 1))
                if r >= S:
                    vector.wait_ge(sa, r - (S - 1))  # ACT freed z slot r-S
                # user half-select: e_u = lo + h * (hi - lo)   (17 wide)
                nc.vector.tensor_sub(d3, gu_hi, gu_lo)
                nc.vector.tensor_mul(sqi3, gi16, gi16)
                nc.vector.tensor_mul(hd3, h173, d3)
                nc.vector.tensor_add(qa3, sqi3, gm_q16)
                nc.vector.tensor_add(eu3, gu_lo, hd3)
                # s17 = e_u + e_i + s_m  (col 16 = total bias term)
                nc.vector.tensor_add(t3, gi17, gm_s17).then_inc(gdone, 1)
                nc.vector.tensor_mul(squ3, eu3[:, :, 0:K], eu3[:, :, 0:K])
                nc.vector.tensor_add(s3, eu3, t3)
                # q16 = e_u^2 + e_i^2 + q_m
                nc.vector.tensor_add(q163, squ3, qa3)
                nc.vector.tensor_mul(s23, s3[:, :, 0:K], s3[:, :, 0:K])
                nc.vector.tensor_sub(dd[:], s2[:], q16[:])
                # DVE pipeline hazard (HW-verified): reduce_sum commits its
                # output near instruction end; a short op right behind it
                # reads stale SBUF.  Self-sem wait forces retirement first.
                nc.vector.reduce_sum(
                    out=rv3, in_=dd3, axis=mybir.AxisListType.X
                ).then_inc(svr, 1)
                vector.wait_ge(svr, r + 1)
                # z = 0.5 * r + bias_term
                nc.vector.scalar_tensor_tensor(
                    out=zv,
                    in0=rv3,
                    scalar=0.5,
                    in1=s3[:, :, K:K + 1],
                    op0=OP.mult,
                    op1=OP.add,
                ).then_inc(sv, 1)

        @block.scalar
        def _(scalar):
            for r in range(repeat):
                s_ = r % S
                scalar.wait_ge(sv, r + 1)
                if r >= S:
                    scalar.wait_ge(outd, 16 * (r - (S - 1)))  # y slot free
                nc.scalar.activation(
                    out=y[:, s_ * T:(s_ + 1) * T],
                    in_=z[:, s_ * T:(s_ + 1) * T],
                    func=AF.Sigmoid,
                ).then_inc(sa, 1)

    nc.compile()
    return nc


def _prep_tables(v, bias):
    """Combined gather table (cached on v/bias identity)."""
    key = (id(v), id(bias))
    hit = _CACHE.get("tables")
    if hit is not None and hit[0] == key:
        return hit[1]
    v = np.asarray(v, dtype=np.float32)
    bias = np.asarray(bias, dtype=np.float32)
    tab = np.zeros((NTAB, ELEM), np.float32)
    vu = v[0:VU].reshape(NU2, 2, K)
    bu = bias[0:VU, 0].reshape(NU2, 2)
    tab[0:NU2, 0:K] = vu[:, 0]
    tab[0:NU2, K] = bu[:, 0]
    tab[0:NU2, 32:32 + K] = vu[:, 1]
    tab[0:NU2, 32 + K] = bu[:, 1]
    tab[OFF_I:OFF_I + VI, 0:K] = v[VU:VU + VI]
    tab[OFF_I:OFF_I + VI, K] = bias[VU:VU + VI, 0]
    vg, vy = v[VU + VI:VU + VI + VG], v[VU + VI + VG:]
    bg, by = bias[VU + VI:VU + VI + VG], bias[VU + VI + VG:]
    tab[OFF_M:, 0:K] = (vg[:, None, :] + vy[None, :, :]).reshape(VM, K)
    tab[OFF_M:, K] = (bg[:, None, 0] + by[None, :, 0]).reshape(VM)
    tab[OFF_M:, K + 1:2 * K + 1] = (
        vg[:, None, :] ** 2 + vy[None, :, :] ** 2
    ).reshape(VM, K)
    _CACHE["tables"] = (key, tab)
    return tab


def _prep_inputs(x, v, bias):
    """Full inputs -> per-core in_maps."""
    x = np.asarray(x)
    tab = _prep_tables(v, bias)
    in_maps = []
    for c in range(N_CORES):
        xc = x[c * ROWS:(c + 1) * ROWS].astype(np.int64)     # (512, 4)
        iu = (xc[:, 0] >> 1)
        h = (xc[:, 0] & 1).astype(np.float32)
        ii = OFF_I + xc[:, 1]
        im = OFF_M + xc[:, 2] * VY + xc[:, 3]
        idx_all = np.concatenate([iu, ii, im]).astype(np.int16)   # [1536]
        idx128 = np.tile(
            np.ascontiguousarray(idx_all.reshape(IDXC, 16).T), (8, 1)
        )                                                         # [128, 96]
        h17 = np.repeat(
            np.ascontiguousarray(h.reshape(T, P).T), 17, axis=1
        ).astype(np.float32)                                      # [128, 68]
        in_maps.append(
            {"idx16": np.ascontiguousarray(idx128), "h17": h17, "tab": tab}
        )
    return in_maps


def _assemble(results):
    """Per-core out[p, t] -> full (BATCH, 1) f32 output."""
    ys = []
    for c in range(N_CORES):
        o = np.asarray(results[c]["out"])                # (P, T)
        ys.append(o.T.reshape(ROWS, 1))                  # row t*128+p
    return np.concatenate(ys, axis=0).astype(np.float32)


def _get_exec(repeat=1):
    """Compile the SPMD program once; returns a cached jitted callable.

    Mirrors the multi-core branch of concourse.bass2jax.run_bass_via_pjrt
    but keeps the jitted function alive so repeat calls skip recompilation.
    """
    key = ("exec", repeat)
    if key in _CACHE:
        return _CACHE[key]
    import jax
    from jax.experimental.shard_map import shard_map
    from jax.sharding import Mesh, PartitionSpec

    import concourse.mybir as mybir
    from concourse import bass2jax

    bass2jax.install_neuronx_cc_hook()
    nc = _build(repeat)
    assert nc.dbg_addr is None
    partition_name = nc.partition_id_tensor.name if nc.partition_id_tensor else None

    in_names, out_names, out_avals = [], [], []
    for alloc in nc.m.functions[0].allocations:
        if not isinstance(alloc, mybir.MemoryLocationSet):
            continue
        name = alloc.memorylocations[0].name
        if alloc.kind == "ExternalInput":
            if name != partition_name:
                in_names.append(name)
        elif alloc.kind == "ExternalOutput":
            out_names.append(name)
            out_avals.append(
                jax.core.ShapedArray(
                    tuple(alloc.tensor_shape), mybir.dt.np(alloc.dtype)
                )
            )
    n_params, n_outs = len(in_names), len(out_names)
    all_names = in_names + out_names + ([partition_name] if partition_name else [])

    def _body(*args):
        operands = list(args)
        if partition_name is not None:
            operands.append(bass2jax.partition_id_tensor())
        outs = bass2jax._bass_exec_p.bind(
            *operands,
            out_avals=tuple(out_avals),
            in_names=tuple(all_names),
            out_names=tuple(out_names),
            lowering_input_output_aliases=(),
            sim_require_finite=True,
            sim_require_nnan=True,
            nc=nc,
        )
        return tuple(outs)

    devices = jax.devices()[:N_CORES]
    mesh = Mesh(np.asarray(devices), ("core",))
    fn = jax.jit(
        shard_map(
            _body,
            mesh=mesh,
            in_specs=(PartitionSpec("core"),) * (n_params + n_outs),
            out_specs=(PartitionSpec("core"),) * n_outs,
            check_rep=False,
        ),
        donate_argnums=tuple(range(n_params, n_params + n_outs)),
        keep_unused=True,
    )
    _CACHE[key] = (fn, in_names, out_names, out_avals, mesh)
    return _CACHE[key]


def _concat_inputs(x, v, bias, in_names):
    in_maps = _prep_inputs(x, v, bias)
    return [
        np.concatenate([in_maps[c][nm] for c in range(N_CORES)], axis=0)
        for nm in in_names
    ]


def _zero_outs(out_avals):
    return [
        np.zeros((N_CORES * av.shape[0], *av.shape[1:]), av.dtype)
        for av in out_avals
    ]


def run(x, v, bias, trace=False):
    """Returns (y, exec_time_ns_or_None)."""
    fn, in_names, out_names, out_avals, _ = _get_exec()
    outs = fn(*_concat_inputs(x, v, bias, in_names), *_zero_outs(out_avals))
    o = np.asarray(outs[out_names.index("out")]).reshape(N_CORES, P, T)
    return _assemble([{"out": o[c]} for c in range(N_CORES)]), None


def bench(x, v, bias, rounds=12, per_round=8, r1=8, r2=512):
    """Per-iteration kernel time via interleaved two-point unroll diff
    (cancels per-call RPC/dispatch overhead and slow drift)."""
    import time

    import jax
    from jax.sharding import NamedSharding, PartitionSpec

    def caller(repeat):
        fn, in_names, out_names, out_avals, mesh = _get_exec(repeat)
        sh = NamedSharding(mesh, PartitionSpec("core"))
        dev_in = [
            jax.device_put(a, sh)
            for a in _concat_inputs(x, v, bias, in_names)
        ]
        zeros = _zero_outs(out_avals)

        def call():
            return fn(*dev_in, *[jax.device_put(zz, sh) for zz in zeros])

        return call

    callA, callB = caller(r1), caller(r2)
    callA()[0].block_until_ready()
    callB()[0].block_until_ready()
    diffs = []
    for _ in range(rounds):
        tA, tB = [], []
        for _ in range(per_round):
            t0 = time.perf_counter()
            callA()[0].block_until_ready()
            tA.append(time.perf_counter() - t0)
        for _ in range(per_round):
            t0 = time.perf_counter()
            callB()[0].block_until_ready()
            tB.append(time.perf_counter() - t0)
        diffs.append(min(tB) - min(tA))
    diffs.sort()
    return diffs[len(diffs) // 2] / (r2 - r1) * 1e9


def kernel(x, v, bias):
    y, _ = run(x, v, bias, trace=False)
    return y
